# revision 55
# baseline (speedup 1.0000x reference)
"""Causal single-head attention (B=4, T=4096, E=1024, H=64) on 8 TRN2 cores.

Sharding: 2 cores per batch; no collectives (host shards, device computes,
host gathers). Queries are assigned to cores in 256-row half-groups with the
fold pattern {0,3}/{1,2} (mod 4), which makes both cores' causal work-lists
IDENTICAL: 8 query slots with key-group trip counts exactly (1..8), so one
SPMD graph serves all cores; all per-core variation (which queries, causal
mask content, key order) lives in host-prepared input data.

Host prep (layout-only, no FLOPs): x[b]^T cast to bf16 with columns permuted
to [owned half-groups in slot order | partner half-groups in the other
core's slot order]. Because each original 512-token key-pair always splits
one-owned/one-foreign, the compiled per-key-group column offsets are
identical on every core; the device re-gathers each key-group as
[own-half | foreign-half], and causal masks (shipped as input data) encode
the per-core key order.

Per-core device graph (bf16 compute, f32 PSUM; key-group-major, PE
software-pipelined so exp(i) overlaps scores(i+1)):
  - Q^T projection (owned tokens, slot order); K^T/V^T projection with
    lhsT=[Wk|Wv] for full 128-wide PE utilization, interleaved with
    attention as each key-group's data lands (DMA/compute overlap)
  - V^T -> V1 [128-token blocks, 65] via PE transpose; column 64 = ones so
    the softmax denominator falls out of the attn@V matmul
  - Adjacent slot pairs share N=512 score/AV matmuls (fewer LDWEIGHTS);
    exp on ACT with scale=E^-0.5 folded in; multiplicative causal mask (DVE)
    on diagonal items only; U^T[65,:] += V1_blk.T @ exp^T accumulated in
    PSUM per item, summed across key-groups in SBUF (DVE)
  - Epilogue per slot: PE-transpose U^T, DVE reciprocal of the denominator
    column, per-partition scale, DMA out f32.
"""
import numpy as np
import ml_dtypes

B, T, E, H = 4, 4096, 1024, 64
HGS = 256         # queries per slot (half-group size)
KG = 512          # keys per key-group
NSLOT = 8
NQ = NSLOT * HGS  # 2048 owned queries per core
ET = E // 128     # 8 E-tiles
NKB = T // 128    # 32 key blocks
SCALE = float(E) ** -0.5

HGS_A = [0, 3, 4, 7, 8, 11, 12, 15]   # core half 0: needs 1..8 in slot order
HGS_B = [1, 2, 5, 6, 9, 10, 13, 14]   # core half 1: needs 1..8 in slot order

_cache = {}


def _bf16(a):
    return np.ascontiguousarray(a.astype(ml_dtypes.bfloat16))


def _build_graph():
    import concourse.mybir as mybir
    import concourse.tile as tile
    from concourse import bacc
    from concourse.masks import make_identity

    dt = mybir.dt
    nc = bacc.Bacc(None, target_bir_lowering=False)
    xT_e = nc.declare_dram_parameter("xT", [E, T], dt.bfloat16, isOutput=False)
    wkv_e = nc.declare_dram_parameter("wkv", [E, 128], dt.bfloat16, isOutput=False)
    wq_e = nc.declare_dram_parameter("wq", [E, H], dt.bfloat16, isOutput=False)
    mask_e = nc.declare_dram_parameter("mask", [128, NSLOT * 4 * HGS], dt.bfloat16,
                                       isOutput=False)
    out_e = nc.declare_dram_parameter("out", [NQ, H], dt.float32, isOutput=True)

    xT_r = xT_e.rearrange("(et p) t -> p et t", p=128)

    with tile.TileContext(nc) as tc:
        with (
            tc.tile_pool(name="singles", bufs=1) as singles,
            tc.tile_pool(name="persist", bufs=1) as persist,
        ):
            identity = singles.tile([128, 128], dt.bfloat16)
            make_identity(nc, identity)
            identity32 = singles.tile([H + 1, H + 1], dt.float32)
            make_identity(nc, identity32)
            wkv_sb = singles.tile([128, ET, 128], dt.bfloat16)
            nc.scalar.dma_start(out=wkv_sb, in_=wkv_e.rearrange("(et p) m -> p et m", p=128))
            wq_sb = singles.tile([128, ET, H], dt.bfloat16)
            nc.scalar.dma_start(out=wq_sb, in_=wq_e.rearrange("(et p) m -> p et m", p=128))
            mask_sb = singles.tile([128, NSLOT * 4 * HGS], dt.bfloat16)

            # persistent activations
            kvT = persist.tile([128, T], dt.bfloat16)    # rows 0:64 K^T, 64:128 V^T
            v1 = persist.tile([128, NKB, H + 1], dt.bfloat16)
            qT = persist.tile([64, NQ], dt.bfloat16)
            u_acc = persist.tile([H + 1, NSLOT, HGS], dt.float32)
            # per pair g: columns [own_2g | foreign_2g | own_2g+1 | foreign_2g+1]
            xq_tiles = [persist.tile([128, ET, 4, HGS], dt.bfloat16,
                                     name=f"xq{g}") for g in range(4)]

            nc.vector.memset(v1[:, :, H], 1.0)  # denominator ones column

            # ---- fused pipeline: proj + attention, key-group-major ----
            with (
                tc.tile_pool(name="xkv", bufs=3) as xkv,
                tc.tile_pool(name="xin", bufs=2) as xin,
                tc.tile_pool(name="pscore", bufs=2, space="PSUM") as pscore,
                tc.tile_pool(name="pproj", bufs=2, space="PSUM") as pproj,
                tc.tile_pool(name="pu", bufs=1, space="PSUM") as pu,
                tc.tile_pool(name="pepi", bufs=1, space="PSUM") as pepi,
                tc.tile_pool(name="ex", bufs=3) as expool,
                tc.tile_pool(name="epi", bufs=4) as epi,
            ):
                def prefetch_kv(s):
                    if s < T // KG:
                        nc.sync.dma_start(
                            out=xq_tiles[s // 2][:, :, 2 * (s % 2) + 1, :],
                            in_=xT_r[:, :, NQ + s * HGS:NQ + (s + 1) * HGS])

                def qdma(g):
                    for two in range(2):
                        nc.sync.dma_start(
                            out=xq_tiles[g][:, :, 2 * two, :],
                            in_=xT_r[:, :, g * KG + two * HGS:
                                     g * KG + (two + 1) * HGS])
                    return g

                def qmm(g, _xg=None):
                    # Two concurrent M=64 accumulation chains on PE column
                    # groups (0,0)/(0,64) — even E-tiles left, odd right —
                    # then sum the halves on DVE. ~2x PE throughput for the
                    # half-width Q projection (tile_position concurrency).
                    own = xq_tiles[g].rearrange(
                        "p et (two fo) c -> p et two fo c", fo=2)[:, :, :, 0, :]
                    ps = pproj.tile([128, KG], dt.float32, tag="pj", name="ps_q")
                    for et in range(ET):
                        col = 64 * (et % 2)
                        nc.tensor.matmul(ps[col:col + 64, :],
                                         lhsT=wq_sb[:, et, :],
                                         rhs=own[:, et, :, :],
                                         tile_position=(0, col),
                                         start=(et < 2), stop=(et >= ET - 2),
                                         skip_group_check=True)
                    qa = epi.tile([64, KG], dt.float32, tag="qa")
                    qb = epi.tile([64, KG], dt.float32, tag="qb")
                    nc.scalar.copy(out=qa, in_=ps[0:64, :])
                    nc.vector.tensor_copy(out=qb, in_=ps[64:128, :])
                    nc.vector.tensor_add(qT[:, g * KG:(g + 1) * KG], qa, qb)

                if True:
                    # pending: (q0 col, width, j, exT, done_slots)
                    pending = []
                    last_exp = [None]

                    def pitem_front(p, j):
                        """Paired item: slots (2p, 2p+1), key-group j, N=512.
                        Masked on slot 2p's half when j == 2p (its diagonal)."""
                        a = 2 * p
                        q_ap = qT[:, a * HGS:(a + 2) * HGS]
                        exT = expool.tile([128, 4, 2 * HGS], dt.bfloat16, tag="ex")
                        for half in range(2):
                            psh = pscore.tile([128, 2, 2 * HGS], dt.float32, tag="sc",
                                              name="ps_h")
                            for rr in range(2):
                                r = 2 * half + rr
                                kb = 4 * j + r
                                nc.tensor.matmul(
                                    psh[:, rr, :],
                                    lhsT=kvT[0:64, kb * 128:(kb + 1) * 128],
                                    rhs=q_ap, start=True, stop=True)
                            last_exp[0] = nc.scalar.activation(
                                out=exT[:, 2 * half:2 * half + 2, :], in_=psh,
                                func=mybir.ActivationFunctionType.Exp, scale=SCALE)
                        if j == a:
                            nc.vector.tensor_mul(
                                exT[:, :, 0:HGS], exT[:, :, 0:HGS],
                                mask_sb[:, a * 4 * HGS:(a + 1) * 4 * HGS]
                                .rearrange("p (r c) -> p r c", r=4))
                        done = [a] if j == a else []
                        pending.append((a, 2, j, exT, done))

                    def sitem_front(b):
                        """Solo diagonal item for odd slot b at key-group j=b."""
                        j = b
                        q_ap = qT[:, b * HGS:(b + 1) * HGS]
                        exT = expool.tile([128, 4, HGS], dt.bfloat16, tag="ex",
                                          name="exs")
                        ps4 = pscore.tile([128, 4, HGS], dt.float32, tag="sc",
                                          name="ps_s")
                        for r in range(4):
                            kb = 4 * j + r
                            nc.tensor.matmul(
                                ps4[:, r, :],
                                lhsT=kvT[0:64, kb * 128:(kb + 1) * 128],
                                rhs=q_ap, start=True, stop=True)
                        nc.scalar.activation(
                            out=exT, in_=ps4,
                            func=mybir.ActivationFunctionType.Exp, scale=SCALE)
                        nc.vector.tensor_mul(
                            exT, exT,
                            mask_sb[:, b * 4 * HGS:(b + 1) * 4 * HGS]
                            .rearrange("p (r c) -> p r c", r=4))
                        pending.append((b, 1, j, exT, [b]))

                    def flush_av():
                        s0, w, j, exT, done = pending.pop(0)
                        u_it = pu.tile([H + 1, 2 * HGS], dt.float32, tag="u")
                        uv = u_it[:, 0:w * HGS]
                        for r in range(4):
                            nc.tensor.matmul(
                                uv, lhsT=v1[:, 4 * j + r, :],
                                rhs=exT[:, r, :],
                                start=(r == 0), stop=(r == 3))
                        acc = u_acc[:, s0, :] if w == 1 else \
                            u_acc[:, s0:s0 + 2, :].rearrange("p a c -> p (a c)")
                        if j == 0:
                            nc.vector.tensor_copy(out=acc, in_=uv)
                        else:
                            nc.vector.tensor_add(acc, acc, uv)
                        for s in done:
                            epilogue(s)

                    def epilogue(s):
                        for hh in range(2):
                            pst = pepi.tile([128, H + 1], dt.float32, tag="tp")
                            nc.tensor.transpose(
                                pst, u_acc[:, s, hh * 128:(hh + 1) * 128],
                                identity32[:, :])
                            rec = epi.tile([128, 1], dt.float32, tag="rec")
                            nc.vector.reciprocal(rec, pst[:, H:H + 1])
                            o_sb = epi.tile([128, H], dt.float32, tag="o")
                            nc.vector.tensor_scalar_mul(o_sb, pst[:, 0:H], rec)
                            row0 = s * HGS + hh * 128
                            nc.scalar.dma_start(out=out_e[row0:row0 + 128, :],
                                                in_=o_sb)

                    def kvproj(j, split=False):
                        xj = xq_tiles[j // 2][:, :, 2 * (j % 2):2 * (j % 2) + 2, :]
                        psp = pproj.tile([128, KG], dt.float32, tag="pj")
                        if split:
                            for two in range(2):
                                for et in range(ET):
                                    nc.tensor.matmul(
                                        psp[:, two * HGS:(two + 1) * HGS],
                                        lhsT=wkv_sb[:, et, :],
                                        rhs=xj[:, et, two, :],
                                        start=(et == 0), stop=(et == ET - 1))
                        else:
                            for et in range(ET):
                                nc.tensor.matmul(
                                    psp, lhsT=wkv_sb[:, et, :],
                                    rhs=xj[:, et, :, :],
                                    start=(et == 0), stop=(et == ET - 1))
                        nc.vector.tensor_copy(out=kvT[:, j * KG:(j + 1) * KG],
                                              in_=psp)
                        for r in range(4):
                            kb = 4 * j + r
                            pst = pepi.tile([128, H + 1], dt.bfloat16, tag="tp",
                                            name="pst_vt")
                            nc.tensor.transpose(
                                pst[:, 0:H], kvT[64:128, kb * 128:(kb + 1) * 128],
                                identity[64:128, 64:128])
                            nc.vector.tensor_copy(out=v1[:, kb, 0:H], in_=pst[:, 0:H])

                    def pitem(p, j):
                        pitem_front(p, j)
                        while len(pending) > 1:
                            flush_av()

                    def sitem(b):
                        sitem_front(b)
                        while len(pending) > 1:
                            flush_av()

                    # step 0: interleave Q projection groups with step-0 items.
                    # DMA ring order: xT0, xq0, mask01, xq1, xT1, xq2, xq3,
                    # xT2 ... so no consumer waits.
                    prefetch_kv(0)
                    xq_t = {0: qdma(0)}
                    kvproj(0)
                    qmm(0, xq_t.pop(0))
                    nc.scalar.dma_start(out=mask_sb[:, 0:2 * 4 * HGS],
                                        in_=mask_e[:, 0:2 * 4 * HGS])
                    xq_t[1] = qdma(1)
                    pitem(0, 0)       # masked on slot 0 (its diagonal)
                    xq_t[2] = qdma(2)
                    xq_t[3] = qdma(3)
                    prefetch_kv(1)
                    qmm(1, xq_t.pop(1))
                    pitem(1, 0)
                    qmm(2, xq_t.pop(2))
                    prefetch_kv(2)
                    pitem(2, 0)
                    qmm(3, xq_t.pop(3))
                    pitem(3, 0)
                    # steps 1..7: paired items for pairs p >= ceil(j/2); odd j
                    # additionally has the solo diagonal of slot j
                    for j in range(1, NSLOT):
                        kvproj(j)
                        if j == 1:
                            m2 = nc.scalar.dma_start(out=mask_sb[:, 2 * 4 * HGS:],
                                                     in_=mask_e[:, 2 * 4 * HGS:])
                            tile.add_dep_helper(last_exp[0].ins, m2.ins,
                                                sync=False,
                                                reason="mask2 after step0 exps")
                        prefetch_kv(j + 2)
                        for p in range((j + 1) // 2, 4):
                            pitem(p, j)
                        if j % 2 == 1:
                            sitem(j)  # diag of odd slot j, emitted last
                    while pending:
                        flush_av()
    nc.compile()
    return nc


def _make_masks(hgs):
    """Diagonal masks for the per-core key order [own hg | partner hg] within
    each key-group: rows 0:256 self-triangle, rows 256:512 all-valid iff the
    own half-group is the later (odd) member of its pair."""
    masks = np.zeros((NSLOT, KG, HGS), dtype=np.float32)
    rk = np.arange(HGS)[:, None]
    cq = np.arange(HGS)[None, :]
    for s, hg in enumerate(hgs):
        masks[s, 0:HGS, :] = (rk <= cq)
        masks[s, HGS:KG, :] = 1.0 if hg % 2 == 1 else 0.0
    # device layout: [partition 128, slot, blockrow 4, col 256]
    m = masks.reshape(NSLOT, 4, 128, HGS).transpose(2, 0, 1, 3)
    return _bf16(m.reshape(128, NSLOT * 4 * HGS))


def kernel(x, Wk, Wq, Wv):
    from concourse.bass_utils import run_bass_kernel_spmd

    x = np.asarray(x, dtype=np.float32)
    Wk = np.asarray(Wk, dtype=np.float32)
    Wq = np.asarray(Wq, dtype=np.float32)
    Wv = np.asarray(Wv, dtype=np.float32)

    if "nc" not in _cache:
        _cache["nc"] = _build_graph()
    nc = _cache["nc"]

    wkv = _bf16(np.concatenate([Wk, Wv], axis=1))
    wq = _bf16(Wq)
    mask_by_half = [_make_masks(HGS_A), _make_masks(HGS_B)]

    in_maps = []
    core_meta = []
    for b in range(B):
        xTb = _bf16(x[b].T)  # [E, T]
        for half, hgs in enumerate([HGS_A, HGS_B]):
            other = [HGS_A, HGS_B][1 - half]
            xp = np.concatenate(
                [xTb[:, hg * HGS:(hg + 1) * HGS] for hg in list(hgs) + other],
                axis=1)
            in_maps.append({
                "xT": np.ascontiguousarray(xp),
                "wkv": wkv,
                "wq": wq,
                "mask": mask_by_half[half],
            })
            core_meta.append((b, hgs))

    res = run_bass_kernel_spmd(nc, in_maps, core_ids=list(range(8)),
                               **_cache.get("run_kwargs", {}))
    _cache["last_result"] = res

    full = np.zeros((B, T, H), dtype=np.float32)
    for core, (b, hgs) in enumerate(core_meta):
        o = res.results[core]["out"]
        for s, hg in enumerate(hgs):
            full[b, hg * HGS:(hg + 1) * HGS, :] = o[s * HGS:(s + 1) * HGS, :]
    return full


# revision 56
# speedup vs baseline: 1.0229x; 1.0229x over previous
"""Causal single-head attention (B=4, T=4096, E=1024, H=64) on 8 TRN2 cores.

Sharding: 2 cores per batch; no collectives (host shards, device computes,
host gathers). Queries are assigned to cores in 256-row half-groups with the
fold pattern {0,3}/{1,2} (mod 4), which makes both cores' causal work-lists
IDENTICAL: 8 query slots with key-group trip counts exactly (1..8), so one
SPMD graph serves all cores; all per-core variation (which queries, causal
mask content, key order) lives in host-prepared input data.

Host prep (layout-only, no FLOPs): x[b]^T cast to bf16 with columns permuted
to [owned half-groups in slot order | partner half-groups in the other
core's slot order]. Because each original 512-token key-pair always splits
one-owned/one-foreign, the compiled per-key-group column offsets are
identical on every core; the device re-gathers each key-group as
[own-half | foreign-half], and causal masks (shipped as input data) encode
the per-core key order.

Per-core device graph (bf16 compute, f32 PSUM; key-group-major, PE
software-pipelined so exp(i) overlaps scores(i+1)):
  - Q^T projection (owned tokens, slot order); K^T/V^T projection with
    lhsT=[Wk|Wv] for full 128-wide PE utilization, interleaved with
    attention as each key-group's data lands (DMA/compute overlap)
  - V^T -> V1 [128-token blocks, 65] via PE transpose; column 64 = ones so
    the softmax denominator falls out of the attn@V matmul
  - Adjacent slot pairs share N=512 score/AV matmuls (fewer LDWEIGHTS);
    exp on ACT with scale=E^-0.5 folded in; multiplicative causal mask (DVE)
    on diagonal items only; U^T[65,:] += V1_blk.T @ exp^T accumulated in
    PSUM per item, summed across key-groups in SBUF (DVE)
  - Epilogue per slot: PE-transpose U^T, DVE reciprocal of the denominator
    column, per-partition scale, DMA out f32.
"""
import numpy as np
import ml_dtypes

B, T, E, H = 4, 4096, 1024, 64
HGS = 256         # queries per slot (half-group size)
KG = 512          # keys per key-group
NSLOT = 8
NQ = NSLOT * HGS  # 2048 owned queries per core
ET = E // 128     # 8 E-tiles
NKB = T // 128    # 32 key blocks
SCALE = float(E) ** -0.5

HGS_A = [0, 3, 4, 7, 8, 11, 12, 15]   # core half 0: needs 1..8 in slot order
HGS_B = [1, 2, 5, 6, 9, 10, 13, 14]   # core half 1: needs 1..8 in slot order

_cache = {}


def _bf16(a):
    return np.ascontiguousarray(a.astype(ml_dtypes.bfloat16))


def _build_graph():
    import concourse.mybir as mybir
    import concourse.tile as tile
    from concourse import bacc
    from concourse.masks import make_identity

    dt = mybir.dt
    nc = bacc.Bacc(None, target_bir_lowering=False)
    xT_e = nc.declare_dram_parameter("xT", [E, T], dt.bfloat16, isOutput=False)
    wkv_e = nc.declare_dram_parameter("wkv", [E, 128], dt.bfloat16, isOutput=False)
    wq_e = nc.declare_dram_parameter("wq", [E, H], dt.bfloat16, isOutput=False)
    mask_e = nc.declare_dram_parameter("mask", [128, NSLOT * 4 * HGS], dt.bfloat16,
                                       isOutput=False)
    out_e = nc.declare_dram_parameter("out", [NQ, H], dt.float32, isOutput=True)

    xT_r = xT_e.rearrange("(et p) t -> p et t", p=128)

    with tile.TileContext(nc) as tc:
        with (
            tc.tile_pool(name="singles", bufs=1) as singles,
            tc.tile_pool(name="persist", bufs=1) as persist,
        ):
            identity = singles.tile([128, 128], dt.bfloat16)
            make_identity(nc, identity)
            identity32 = singles.tile([H + 1, H + 1], dt.float32)
            make_identity(nc, identity32)
            wkv_sb = singles.tile([128, ET, 128], dt.bfloat16)
            nc.scalar.dma_start(out=wkv_sb, in_=wkv_e.rearrange("(et p) m -> p et m", p=128))
            wq_sb = singles.tile([128, ET, H], dt.bfloat16)
            nc.scalar.dma_start(out=wq_sb, in_=wq_e.rearrange("(et p) m -> p et m", p=128))
            mask_sb = singles.tile([128, NSLOT * 4 * HGS], dt.bfloat16)

            # persistent activations
            kvT = persist.tile([128, T], dt.bfloat16)    # rows 0:64 K^T, 64:128 V^T
            v1 = persist.tile([128, NKB, H + 1], dt.bfloat16)
            qT = persist.tile([64, NQ], dt.bfloat16)
            u_acc = persist.tile([H + 1, NSLOT, HGS], dt.float32)
            # per pair g: columns [own_2g | foreign_2g | own_2g+1 | foreign_2g+1]
            xq_tiles = [persist.tile([128, ET, 4, HGS], dt.bfloat16,
                                     name=f"xq{g}") for g in range(4)]

            nc.vector.memset(v1[:, :, H], 1.0)  # denominator ones column

            # ---- fused pipeline: proj + attention, key-group-major ----
            with (
                tc.tile_pool(name="xkv", bufs=3) as xkv,
                tc.tile_pool(name="xin", bufs=2) as xin,
                tc.tile_pool(name="pscore", bufs=2, space="PSUM") as pscore,
                tc.tile_pool(name="pproj", bufs=2, space="PSUM") as pproj,
                tc.tile_pool(name="pu", bufs=1, space="PSUM") as pu,
                tc.tile_pool(name="pepi", bufs=1, space="PSUM") as pepi,
                tc.tile_pool(name="ex", bufs=3) as expool,
                tc.tile_pool(name="epi", bufs=4) as epi,
            ):
                def prefetch_kv(s):
                    if s < T // KG:
                        nc.sync.dma_start(
                            out=xq_tiles[s // 2][:, :, 2 * (s % 2) + 1, :],
                            in_=xT_r[:, :, NQ + s * HGS:NQ + (s + 1) * HGS])

                def qdma(g):
                    for two in range(2):
                        nc.sync.dma_start(
                            out=xq_tiles[g][:, :, 2 * two, :],
                            in_=xT_r[:, :, g * KG + two * HGS:
                                     g * KG + (two + 1) * HGS])
                    return g

                def qmm(g, _xg=None):
                    own = xq_tiles[g].rearrange(
                        "p et (two fo) c -> p et two fo c", fo=2)[:, :, :, 0, :]
                    ps = pproj.tile([128, KG], dt.float32, tag="pj", name="ps_q")
                    if g == 0:
                        # group 0 gates the first attention item: use the
                        # shortest-latency single chain + direct copy
                        for et in range(ET):
                            nc.tensor.matmul(ps[0:64, :], lhsT=wq_sb[:, et, :],
                                             rhs=own[:, et, :, :],
                                             start=(et == 0),
                                             stop=(et == ET - 1))
                        nc.scalar.copy(out=qT[:, 0:KG], in_=ps[0:64, :])
                        return
                    # groups 1-3: two concurrent M=64 chains on PE column
                    # groups (0,0)/(0,64) (even/odd E-tiles), summed on DVE —
                    # ~2x PE throughput via tile_position concurrency.
                    for et in range(ET):
                        col = 64 * (et % 2)
                        nc.tensor.matmul(ps[col:col + 64, :],
                                         lhsT=wq_sb[:, et, :],
                                         rhs=own[:, et, :, :],
                                         tile_position=(0, col),
                                         start=(et < 2), stop=(et >= ET - 2),
                                         skip_group_check=True)
                    qa = epi.tile([64, KG], dt.float32, tag="qa")
                    qb = epi.tile([64, KG], dt.float32, tag="qb")
                    nc.scalar.copy(out=qa, in_=ps[0:64, :])
                    nc.vector.tensor_copy(out=qb, in_=ps[64:128, :])
                    nc.vector.tensor_add(qT[:, g * KG:(g + 1) * KG], qa, qb)

                if True:
                    # pending: (q0 col, width, j, exT, done_slots)
                    pending = []
                    last_exp = [None]

                    def pitem_front(p, j):
                        """Paired item: slots (2p, 2p+1), key-group j, N=512.
                        Masked on slot 2p's half when j == 2p (its diagonal)."""
                        a = 2 * p
                        q_ap = qT[:, a * HGS:(a + 2) * HGS]
                        exT = expool.tile([128, 4, 2 * HGS], dt.bfloat16, tag="ex")
                        for half in range(2):
                            psh = pscore.tile([128, 2, 2 * HGS], dt.float32, tag="sc",
                                              name="ps_h")
                            for rr in range(2):
                                r = 2 * half + rr
                                kb = 4 * j + r
                                nc.tensor.matmul(
                                    psh[:, rr, :],
                                    lhsT=kvT[0:64, kb * 128:(kb + 1) * 128],
                                    rhs=q_ap, start=True, stop=True)
                            last_exp[0] = nc.scalar.activation(
                                out=exT[:, 2 * half:2 * half + 2, :], in_=psh,
                                func=mybir.ActivationFunctionType.Exp, scale=SCALE)
                        if j == a:
                            nc.vector.tensor_mul(
                                exT[:, :, 0:HGS], exT[:, :, 0:HGS],
                                mask_sb[:, a * 4 * HGS:(a + 1) * 4 * HGS]
                                .rearrange("p (r c) -> p r c", r=4))
                        done = [a] if j == a else []
                        pending.append((a, 2, j, exT, done))

                    def sitem_front(b):
                        """Solo diagonal item for odd slot b at key-group j=b."""
                        j = b
                        q_ap = qT[:, b * HGS:(b + 1) * HGS]
                        exT = expool.tile([128, 4, HGS], dt.bfloat16, tag="ex",
                                          name="exs")
                        ps4 = pscore.tile([128, 4, HGS], dt.float32, tag="sc",
                                          name="ps_s")
                        for r in range(4):
                            kb = 4 * j + r
                            nc.tensor.matmul(
                                ps4[:, r, :],
                                lhsT=kvT[0:64, kb * 128:(kb + 1) * 128],
                                rhs=q_ap, start=True, stop=True)
                        nc.scalar.activation(
                            out=exT, in_=ps4,
                            func=mybir.ActivationFunctionType.Exp, scale=SCALE)
                        nc.vector.tensor_mul(
                            exT, exT,
                            mask_sb[:, b * 4 * HGS:(b + 1) * 4 * HGS]
                            .rearrange("p (r c) -> p r c", r=4))
                        pending.append((b, 1, j, exT, [b]))

                    def flush_av():
                        s0, w, j, exT, done = pending.pop(0)
                        u_it = pu.tile([H + 1, 2 * HGS], dt.float32, tag="u")
                        uv = u_it[:, 0:w * HGS]
                        for r in range(4):
                            nc.tensor.matmul(
                                uv, lhsT=v1[:, 4 * j + r, :],
                                rhs=exT[:, r, :],
                                start=(r == 0), stop=(r == 3))
                        acc = u_acc[:, s0, :] if w == 1 else \
                            u_acc[:, s0:s0 + 2, :].rearrange("p a c -> p (a c)")
                        if j == 0:
                            nc.vector.tensor_copy(out=acc, in_=uv)
                        else:
                            nc.vector.tensor_add(acc, acc, uv)
                        for s in done:
                            epilogue(s)

                    def epilogue(s):
                        for hh in range(2):
                            pst = pepi.tile([128, H + 1], dt.float32, tag="tp")
                            nc.tensor.transpose(
                                pst, u_acc[:, s, hh * 128:(hh + 1) * 128],
                                identity32[:, :])
                            rec = epi.tile([128, 1], dt.float32, tag="rec")
                            nc.vector.reciprocal(rec, pst[:, H:H + 1])
                            o_sb = epi.tile([128, H], dt.float32, tag="o")
                            nc.vector.tensor_scalar_mul(o_sb, pst[:, 0:H], rec)
                            row0 = s * HGS + hh * 128
                            nc.scalar.dma_start(out=out_e[row0:row0 + 128, :],
                                                in_=o_sb)

                    def kvproj(j, split=False):
                        xj = xq_tiles[j // 2][:, :, 2 * (j % 2):2 * (j % 2) + 2, :]
                        psp = pproj.tile([128, KG], dt.float32, tag="pj")
                        if split:
                            for two in range(2):
                                for et in range(ET):
                                    nc.tensor.matmul(
                                        psp[:, two * HGS:(two + 1) * HGS],
                                        lhsT=wkv_sb[:, et, :],
                                        rhs=xj[:, et, two, :],
                                        start=(et == 0), stop=(et == ET - 1))
                        else:
                            for et in range(ET):
                                nc.tensor.matmul(
                                    psp, lhsT=wkv_sb[:, et, :],
                                    rhs=xj[:, et, :, :],
                                    start=(et == 0), stop=(et == ET - 1))
                        nc.vector.tensor_copy(out=kvT[:, j * KG:(j + 1) * KG],
                                              in_=psp)
                        for r in range(4):
                            kb = 4 * j + r
                            pst = pepi.tile([128, H + 1], dt.bfloat16, tag="tp",
                                            name="pst_vt")
                            nc.tensor.transpose(
                                pst[:, 0:H], kvT[64:128, kb * 128:(kb + 1) * 128],
                                identity[64:128, 64:128])
                            nc.vector.tensor_copy(out=v1[:, kb, 0:H], in_=pst[:, 0:H])

                    def pitem(p, j):
                        pitem_front(p, j)
                        while len(pending) > 1:
                            flush_av()

                    def sitem(b):
                        sitem_front(b)
                        while len(pending) > 1:
                            flush_av()

                    # step 0: interleave Q projection groups with step-0 items.
                    # DMA ring order: xT0, xq0, mask01, xq1, xT1, xq2, xq3,
                    # xT2 ... so no consumer waits.
                    prefetch_kv(0)
                    xq_t = {0: qdma(0)}
                    kvproj(0)
                    qmm(0, xq_t.pop(0))
                    nc.scalar.dma_start(out=mask_sb[:, 0:2 * 4 * HGS],
                                        in_=mask_e[:, 0:2 * 4 * HGS])
                    xq_t[1] = qdma(1)
                    pitem(0, 0)       # masked on slot 0 (its diagonal)
                    xq_t[2] = qdma(2)
                    xq_t[3] = qdma(3)
                    prefetch_kv(1)
                    qmm(1, xq_t.pop(1))
                    pitem(1, 0)
                    qmm(2, xq_t.pop(2))
                    prefetch_kv(2)
                    pitem(2, 0)
                    qmm(3, xq_t.pop(3))
                    pitem(3, 0)
                    # steps 1..7: paired items for pairs p >= ceil(j/2); odd j
                    # additionally has the solo diagonal of slot j
                    for j in range(1, NSLOT):
                        kvproj(j)
                        if j == 1:
                            m2 = nc.scalar.dma_start(out=mask_sb[:, 2 * 4 * HGS:],
                                                     in_=mask_e[:, 2 * 4 * HGS:])
                            tile.add_dep_helper(last_exp[0].ins, m2.ins,
                                                sync=False,
                                                reason="mask2 after step0 exps")
                        prefetch_kv(j + 2)
                        for p in range((j + 1) // 2, 4):
                            pitem(p, j)
                        if j % 2 == 1:
                            sitem(j)  # diag of odd slot j, emitted last
                    while pending:
                        flush_av()
    nc.compile()
    return nc


def _make_masks(hgs):
    """Diagonal masks for the per-core key order [own hg | partner hg] within
    each key-group: rows 0:256 self-triangle, rows 256:512 all-valid iff the
    own half-group is the later (odd) member of its pair."""
    masks = np.zeros((NSLOT, KG, HGS), dtype=np.float32)
    rk = np.arange(HGS)[:, None]
    cq = np.arange(HGS)[None, :]
    for s, hg in enumerate(hgs):
        masks[s, 0:HGS, :] = (rk <= cq)
        masks[s, HGS:KG, :] = 1.0 if hg % 2 == 1 else 0.0
    # device layout: [partition 128, slot, blockrow 4, col 256]
    m = masks.reshape(NSLOT, 4, 128, HGS).transpose(2, 0, 1, 3)
    return _bf16(m.reshape(128, NSLOT * 4 * HGS))


def kernel(x, Wk, Wq, Wv):
    from concourse.bass_utils import run_bass_kernel_spmd

    x = np.asarray(x, dtype=np.float32)
    Wk = np.asarray(Wk, dtype=np.float32)
    Wq = np.asarray(Wq, dtype=np.float32)
    Wv = np.asarray(Wv, dtype=np.float32)

    if "nc" not in _cache:
        _cache["nc"] = _build_graph()
    nc = _cache["nc"]

    wkv = _bf16(np.concatenate([Wk, Wv], axis=1))
    wq = _bf16(Wq)
    mask_by_half = [_make_masks(HGS_A), _make_masks(HGS_B)]

    in_maps = []
    core_meta = []
    for b in range(B):
        xTb = _bf16(x[b].T)  # [E, T]
        for half, hgs in enumerate([HGS_A, HGS_B]):
            other = [HGS_A, HGS_B][1 - half]
            xp = np.concatenate(
                [xTb[:, hg * HGS:(hg + 1) * HGS] for hg in list(hgs) + other],
                axis=1)
            in_maps.append({
                "xT": np.ascontiguousarray(xp),
                "wkv": wkv,
                "wq": wq,
                "mask": mask_by_half[half],
            })
            core_meta.append((b, hgs))

    res = run_bass_kernel_spmd(nc, in_maps, core_ids=list(range(8)),
                               **_cache.get("run_kwargs", {}))
    _cache["last_result"] = res

    full = np.zeros((B, T, H), dtype=np.float32)
    for core, (b, hgs) in enumerate(core_meta):
        o = res.results[core]["out"]
        for s, hg in enumerate(hgs):
            full[b, hg * HGS:(hg + 1) * HGS, :] = o[s * HGS:(s + 1) * HGS, :]
    return full
